# revision 62
# baseline (speedup 1.0000x reference)
"""Barycentric-coordinates KNN kernel for Trainium2 (8 NeuronCores).

Pipeline (per core = one (batch, half-of-V) pair; 8 cores cover 4 batches x 2 halves):
  Phase 1 (device): negated squared distances via TensorE matmul rows
    [-2q,1]x[p,|p|^2] fused with ACT bias/negate; per-64-column-chunk top-8
    values+indices via DVE max8/max_index -> 512 candidates per query row.
  Host: exact top-33 merge (value desc, index asc), neighbor-coordinate
    gather, SHOT weight normalization (no per-partition gather exists on-chip).
  Phase 2 (device): weighted 3x3 covariance (fused multiply-accumulate),
    closed-form eigensolver (Newton on the characteristic cubic + cross
    products), SHOT sign disambiguation, tangent-plane log map, template-cell
    nearest-3 selection via bit-packed keys (dist^2 mantissa | k-slot) and
    max8, onehot payload extraction, barycentric weights.
  Host: decode k-slots from packed keys, pidx = nbr_idx[closest], assemble
    (4, 4096, 5, 8, 3, 2) output.
"""
import sys

sys.path.insert(0, "/opt/trn_rl_repo")

import numpy as np
from contextlib import ExitStack

import concourse.bass as bass
import concourse.mybir as mybir
import concourse.tile as tile
from concourse.bass_utils import run_bass_kernel_spmd
from concourse.tile import ScopedClock

f32 = np.float32
AF = mybir.ActivationFunctionType
ALU = mybir.AluOpType
DT = mybir.dt

B, V, K = 4, 4096, 32
HALF = V // 2            # queries per core
NT = HALF // 128         # 16 v-tiles per core
NCHUNK = 64              # phase-1 chunk count (chunk width 64)
CAND = NCHUNK * 8        # 512 candidates per row
R, A = 5, 8
NCELL = R * A            # 40 template cells
EPS = 1e-8

# ---------------------------------------------------------------------------
# Tile-framework workaround: walrus rejects instructions carrying more than a
# couple of sync waits. Spread extras across single-wait NOPs.
# ---------------------------------------------------------------------------


def _patched_drain_and_barrier(self, tick_clock, wait_clock):
    probe = self.nc.sync.nop(nofuse=True)
    wait_clock.add_sem_waits(probe.ins, ScopedClock({None: tick_clock.global_clock}))
    sync_info = probe.ins.sync_info
    waits = list(sync_info.on_wait or []) if sync_info is not None else []
    if len(waits) > 1:
        sync_info.on_wait = waits[:1]
        for i in range(1, len(waits)):
            extra = self.nc.sync.nop(nofuse=True)
            if extra.ins.sync_info is None:
                extra.ins.sync_info = mybir.SyncInfo(on_wait=[waits[i]], on_update=[])
            else:
                extra.ins.sync_info.on_wait = [waits[i]]
    self.nc.sync.drain()
    self.nc.all_engine_barrier()
    assert self.sems is not None
    popped = self.nc._tile_sem_poison_stack.pop()
    assert popped is self._sem_poison
    self.nc.clear_and_free_semaphores(list(self.sems.allocated().values()))
    self.nc.all_engine_barrier()


tile.TileContext._drain_and_barrier = _patched_drain_and_barrier


def split_sync_waits(nc, max_waits=1):
    for f in nc.m.functions:
        for b in f.blocks:
            new_list = []
            dirty = False
            for ins in b.instructions:
                si = ins.sync_info
                waits = list(si.on_wait) if (si is not None and si.on_wait) else []
                if len(waits) > max_waits:
                    dirty = True
                    extras, keep = waits[:-max_waits], waits[-max_waits:]
                    for j in range(0, len(extras), max_waits):
                        nop = mybir.InstNoOp(
                            name=f"I-wsplit-{nc.next_id()}", engine=ins.engine
                        )
                        nop.sync_info = mybir.SyncInfo(
                            on_wait=extras[j : j + max_waits], on_update=[]
                        )
                        new_list.append(nop)
                    si.on_wait = keep
                new_list.append(ins)
            if dirty:
                b.instructions = new_list


# ---------------------------------------------------------------------------
# Phase 1 program
# ---------------------------------------------------------------------------


NCH1 = 32                # phase-1 chunk count (chunk width 128)
CAND1 = NCH1 * 8         # 256 candidates per row


def build_phase1():
    # d2 = |p|^2 - 2 q.p + |q|^2 via an 11-row fp16 hi/lo-split GEMM (full PE
    # rate; |d2 err| ~1e-6), Relu-clamped, then bit-packed keys
    # (d2 & ~0x7F) | 0x80000000 | local7  so one max8 per 128-chunk yields the
    # 8 nearest (value asc, local idx asc) as negative floats.
    nc = bass.Bass()
    ptm = nc.declare_dram_parameter("ptm", [11, V], DT.float16, isOutput=False)
    qtm = nc.declare_dram_parameter("qtm", [11, HALF], DT.float16, isOutput=False)
    q2v = nc.declare_dram_parameter("q2v", [128, NT], DT.float32, isOutput=False)
    cand_o = nc.declare_dram_parameter("cand", [HALF, CAND1], DT.float32, isOutput=True)

    with tile.TileContext(nc) as tc, ExitStack() as ctx:
        cpool = ctx.enter_context(tc.tile_pool(name="const", bufs=1))
        dpool = ctx.enter_context(tc.tile_pool(name="d2", bufs=3))
        kpool = ctx.enter_context(tc.tile_pool(name="key", bufs=3))
        opool = ctx.enter_context(tc.tile_pool(name="cand", bufs=4))
        ppool = ctx.enter_context(tc.tile_pool(name="psum", bufs=2, space="PSUM"))

        pt = cpool.tile([11, V], DT.float16)
        qt = cpool.tile([11, HALF], DT.float16)
        nv = cpool.tile([128, NT], DT.float32)
        IOTA = cpool.tile([128, V], DT.int32)
        M7 = cpool.tile([128, 1], DT.int32)
        nc.sync.dma_start(pt[:], ptm[:])
        nc.sync.dma_start(qt[:], qtm[:])
        nc.sync.dma_start(nv[:], q2v[:])
        nc.gpsimd.iota(IOTA[:], pattern=[[0, NCH1], [1, 128]], base=-2147483648,
                       channel_multiplier=0)
        nc.vector.memset(M7[:], -128)

        for t in range(NT):
            cand = opool.tile([128, CAND1], DT.float32, tag="cand")
            for jh in range(2):
                d2 = dpool.tile([128, 2048], DT.float32, tag=f"d2{jh}")
                ps = ppool.tile([128, 2048], DT.float32, space="PSUM")
                for k4 in range(4):
                    nc.tensor.matmul(
                        ps[:, k4 * 512:(k4 + 1) * 512],
                        qt[:, t * 128:(t + 1) * 128],
                        pt[:, jh * 2048 + k4 * 512: jh * 2048 + (k4 + 1) * 512],
                        start=True, stop=True,
                    )
                nc.scalar.activation(
                    d2[:], ps[:], AF.Relu, bias=nv[:, t:t + 1], scale=1.0,
                )
                nkey = kpool.tile([128, 2048], DT.float32, tag=f"nkey{jh}")
                nc.vector.scalar_tensor_tensor(
                    out=nkey[:].bitcast(DT.int32), in0=d2[:].bitcast(DT.int32),
                    scalar=M7[:], in1=IOTA[:, jh * 2048:(jh + 1) * 2048],
                    op0=ALU.bitwise_and, op1=ALU.bitwise_or)
                for c in range(NCH1 // 2):
                    co = jh * (NCH1 // 2) + c
                    nc.vector.max(out=cand[:, co * 8:(co + 1) * 8],
                                  in_=nkey[:, c * 128:(c + 1) * 128])
            nc.sync.dma_start(cand_o[t * 128:(t + 1) * 128, :], cand[:])

    split_sync_waits(nc)
    return nc


# ---------------------------------------------------------------------------
# Phase 2 program
# ---------------------------------------------------------------------------


def _register_consts(nc, values):
    for value in values:
        t = nc.alloc_sbuf_tensor(f"const-float32-{value}", [128, 1], DT.float32)
        nc.gpsimd.memset(t.ap(), value)
        nc.const_aps.aps[(DT.float32, value)] = t.ap()
    nc.all_engine_barrier()


PIH = 1.5707963
B2C = -2.6179939  # 2π/3 + π/2 - 2π, keeps the Sin argument within [-π, π]


def build_phase2():
    nc = bass.Bass()
    import os as _os
    if int(_os.environ.get("P2_C1", "0")):
        _register_consts(nc, [0.5])
    else:
        _register_consts(nc, [0.5, PIH, B2C])
    ngh_i = nc.declare_dram_parameter("ngh", [HALF, 96], DT.float32, isOutput=False)
    nw_i = nc.declare_dram_parameter("nw", [HALF, 96], DT.float32, isOutput=False)
    dd_i = nc.declare_dram_parameter("dd", [HALF, K], DT.float32, isOutput=False)
    txy_i = nc.declare_dram_parameter("txy", [128, 2 * NCELL], DT.float32, isOutput=False)
    m3_o = nc.declare_dram_parameter("m3o", [3, HALF, NCELL], DT.float32, isOutput=True)
    px_o = nc.declare_dram_parameter("pxo", [HALF, K], DT.float32, isOutput=True)
    py_o = nc.declare_dram_parameter("pyo", [HALF, K], DT.float32, isOutput=True)

    import os
    _stub = int(os.environ.get("P2_STUB", "0"))
    with tile.TileContext(nc) as tc, ExitStack() as ctx:
        cp = ctx.enter_context(tc.tile_pool(name="const", bufs=1))
        sp = ctx.enter_context(tc.tile_pool(name="scratch", bufs=2))
        bp = ctx.enter_context(tc.tile_pool(name="bc", bufs=2))

        NGH = cp.tile([128, NT, 96], DT.float32)
        NW = cp.tile([128, NT, 96], DT.float32)
        DD = cp.tile([128, NT, K], DT.float32)
        TXY = cp.tile([128, 2 * NCELL], DT.float32)
        nc.sync.dma_start(NGH[:], ngh_i[:].rearrange("(t p) c -> p t c", p=128))
        nc.sync.dma_start(NW[:], nw_i[:].rearrange("(t p) c -> p t c", p=128))
        nc.sync.dma_start(DD[:], dd_i[:].rearrange("(t p) c -> p t c", p=128))
        nc.sync.dma_start(TXY[:], txy_i[:])

        KIOTA = cp.tile([128, NCELL, K], DT.int32)
        nc.gpsimd.iota(KIOTA[:], pattern=[[0, NCELL], [1, K]], base=-2147483648,
                       channel_multiplier=0)
        NEGB = cp.tile([128, 2, NCELL, K], DT.float32)
        nc.vector.memset(NEGB[:], -1e30)
        KIOTA2 = cp.tile([128, 2, NCELL, K], DT.int32)
        nc.vector.tensor_copy(
            KIOTA2[:], KIOTA[:].rearrange("p c k -> p () c k")
            .to_broadcast([128, 2, NCELL, K]))
        M32 = cp.tile([128, 1], DT.int32)
        nc.vector.memset(M32[:], -32)
        # per-(cell,k) constants tx, ty replicated over k and over a 2-tile
        # pair dim (the D stage processes vertex tiles in pairs)
        shp = [128, 2, NCELL, K]
        TXP4 = cp.tile(shp, DT.float32)
        TYP4 = cp.tile(shp, DT.float32)
        TB0 = cp.tile([128, NCELL, K], DT.float32)
        for dst, lo in ((TXP4, 0), (TYP4, NCELL)):
            nc.vector.tensor_copy(
                TB0[:], TXY[:, lo:lo + NCELL].rearrange("p c -> p c ()")
                .to_broadcast([128, NCELL, K]))
            nc.vector.tensor_copy(
                dst[:], TB0[:].rearrange("p c k -> p () c k").to_broadcast(shp))

        _tagn = [0]

        def nt_tile(pool=cp):
            _tagn[0] += 1
            return pool.tile([128, NT], DT.float32, tag=f"nt{_tagn[0]}",
                             name=f"nt{_tagn[0]}")

        # ---- covariance accumulation (one mult + one segmented reduce per
        # pair, batched over all tiles) ----
        CXX, CXY, CXZ, CYY, CYZ, CZZ = [nt_tile() for _ in range(6)]
        cov_dsts = {"xx": CXX, "xy": CXY, "xz": CXZ, "yy": CYY, "yz": CYZ, "zz": CZZ}
        pairs = [("xx", 0, 0), ("xy", 0, 1), ("xz", 0, 2),
                 ("yy", 1, 1), ("yz", 1, 2), ("zz", 2, 2)]
        for nmq, a, b in (pairs if _stub < 4 else []):
            cove = sp.tile([128, NT, K], DT.float32, tag="cove")
            nc.vector.tensor_tensor(out=cove[:], in0=NGH[:, :, a * K:(a + 1) * K],
                                    in1=NW[:, :, b * K:(b + 1) * K], op=ALU.mult)
            nc.vector.tensor_reduce(out=cov_dsts[nmq][:], in_=cove[:],
                                    axis=mybir.AxisListType.X, op=ALU.add)
        if _stub >= 4:
            for i, tl in enumerate((CXX, CXY, CXZ, CYY, CYZ, CZZ)):
                nc.vector.memset(tl[:], 0.1 * (i + 1))

        # ---- eigensolver on (128, NT) ----
        def _ap(x):
            return x if isinstance(x, bass.AP) else x[:]

        def tt(dst, a, bb, op):
            nc.vector.tensor_tensor(out=_ap(dst), in0=_ap(a), in1=_ap(bb), op=op)

        def sq_act(dst, a):
            nc.scalar.activation(_ap(dst), _ap(a), AF.Square)

        Q = nt_tile()
        tt(Q, CXX, CYY, ALU.add)
        tt(Q, Q, CZZ, ALU.add)
        nc.vector.tensor_scalar_mul(Q[:], Q[:], 1.0 / 3.0)
        BXX, BYY, BZZ = nt_tile(), nt_tile(), nt_tile()
        tt(BXX, CXX, Q, ALU.subtract)
        tt(BYY, CYY, Q, ALU.subtract)
        tt(BZZ, CZZ, Q, ALU.subtract)
        P2 = nt_tile()
        T1 = nt_tile(sp)
        sq_act(P2, BXX)
        sq_act(T1, BYY)
        tt(P2, P2, T1, ALU.add)
        sq_act(T1, BZZ)
        tt(P2, P2, T1, ALU.add)
        T2 = nt_tile(sp)
        sq_act(T1, CXY)
        sq_act(T2, CXZ)
        tt(T1, T1, T2, ALU.add)
        sq_act(T2, CYZ)
        tt(T1, T1, T2, ALU.add)
        nc.vector.tensor_scalar_mul(T1[:], T1[:], 2.0)
        tt(P2, P2, T1, ALU.add)
        PP = nt_tile()
        PPX = nt_tile()
        nc.vector.tensor_scalar_mul(PPX[:], P2[:], 1.0 / 6.0)

        def polished_sqrt(dst, x, tmp):
            # ACT Sqrt is ~7e-6; one Newton step s' = (s + x/s)/2 fixes it
            nc.scalar.activation(dst[:], x[:], AF.Sqrt)
            nc.vector.tensor_scalar_max(tmp[:], dst[:], 1e-30)
            nc.vector.reciprocal(tmp[:], tmp[:])
            nc.vector.tensor_tensor(out=tmp[:], in0=x[:], in1=tmp[:], op=ALU.mult)
            nc.vector.tensor_tensor(out=dst[:], in0=dst[:], in1=tmp[:], op=ALU.add)
            nc.vector.tensor_scalar_mul(dst[:], dst[:], 0.5)

        polished_sqrt(PP, PPX, T2)
        PINV = nt_tile()
        nc.vector.tensor_scalar_max(PINV[:], PP[:], 1e-20)
        nc.vector.reciprocal(PINV[:], PINV[:])
        NBXX, NBYY, NBZZ, NBXY, NBXZ, NBYZ = [nt_tile() for _ in range(6)]
        tt(NBXX, BXX, PINV, ALU.mult)
        tt(NBYY, BYY, PINV, ALU.mult)
        tt(NBZZ, BZZ, PINV, ALU.mult)
        tt(NBXY, CXY, PINV, ALU.mult)
        tt(NBXZ, CXZ, PINV, ALU.mult)
        tt(NBYZ, CYZ, PINV, ALU.mult)
        # det(B̂)
        DET = nt_tile()
        sq_act(T1, NBYZ)                     # byz^2
        tt(T2, NBYY, NBZZ, ALU.mult)
        tt(T2, T2, T1, ALU.subtract)
        tt(DET, NBXX, T2, ALU.mult)          # + bxx (byy bzz - byz^2)
        tt(T1, NBXY, NBZZ, ALU.mult)
        tt(T2, NBYZ, NBXZ, ALU.mult)
        tt(T1, T1, T2, ALU.subtract)
        tt(T1, NBXY, T1, ALU.mult)
        tt(DET, DET, T1, ALU.subtract)       # - bxy (bxy bzz - byz bxz)
        tt(T1, NBXY, NBYZ, ALU.mult)
        tt(T2, NBYY, NBXZ, ALU.mult)
        tt(T1, T1, T2, ALU.subtract)
        tt(T1, NBXZ, T1, ALU.mult)
        tt(DET, DET, T1, ALU.add)            # + bxz (bxy byz - byy bxz)
        # ---- closed-form roots: β_k = 2 cos((acos(r) + 2πk)/3), via
        # acos(r) = π/2 - arctan(r / sqrt(1 - r²)) and cos(x) = sin(x + π/2).
        # Roots + eigenvectors batched on (128, 2, NT):
        # index 0 -> lambda_max / x-axis, index 1 -> lambda_min / z-axis
        S2 = [128, 2, NT]

        def nt2_tile(pool=cp):
            _tagn[0] += 1
            return pool.tile(S2, DT.float32, tag=f"nt{_tagn[0]}",
                             name=f"nt{_tagn[0]}")

        def rep2(src):
            d = nt2_tile()
            nc.vector.tensor_copy(
                d[:], src[:].rearrange("p t -> p () t").to_broadcast(S2))
            return d

        R2 = nt_tile()                       # 2r = det  clamped to [-2, 2]
        nc.vector.tensor_scalar_min(R2[:], DET[:], 2.0)
        nc.vector.tensor_scalar_max(R2[:], R2[:], -2.0)
        R22 = rep2(R2)
        BETA = nt2_tile()
        nc.vector.memset(BETA[:, 0, :], 2.2)
        nc.vector.memset(BETA[:, 1, :], -2.2)
        B2T = BETA
        T21 = nt2_tile(sp)
        FV = nt2_tile(sp)
        B2 = nt2_tile(sp)
        R22b = R22
        for _ in range(7):
            sq_act(B2, BETA)
            tt(FV, B2, BETA, ALU.mult)
            nc.vector.scalar_tensor_tensor(
                out=T21[:], in0=BETA[:], scalar=3.0, in1=FV[:],
                op0=ALU.mult, op1=ALU.subtract)
            tt(T21, T21, R22b, ALU.add)
            nc.vector.tensor_scalar(out=B2[:], in0=B2[:], scalar1=3.0,
                                    scalar2=-3.0, op0=ALU.mult, op1=ALU.add)
            nc.vector.tensor_scalar_max(B2[:], B2[:], 1e-8)
            nc.vector.reciprocal(B2[:], B2[:])
            tt(T21, T21, B2, ALU.mult)
            tt(BETA, BETA, T21, ALU.add)
        LAM = nt2_tile()
        PP2 = rep2(PP)
        Q2 = rep2(Q)
        tt(LAM, PP2, BETA, ALU.mult)
        tt(LAM, LAM, Q2, ALU.add)

        C2 = {nm: rep2(src) for nm, src in
              (("xx", CXX), ("xy", CXY), ("xz", CXZ),
               ("yy", CYY), ("yz", CYZ), ("zz", CZZ))}

        def evec2(lam):
            # columns of A - lam I
            NLAM = nt2_tile(sp)
            nc.vector.tensor_scalar_mul(NLAM[:], lam[:], -1.0)
            D0, D1, D2 = nt2_tile(sp), nt2_tile(sp), nt2_tile(sp)
            tt(D0, C2["xx"], NLAM, ALU.add)
            tt(D1, C2["yy"], NLAM, ALU.add)
            tt(D2, C2["zz"], NLAM, ALU.add)
            m0 = (D0, C2["xy"], C2["xz"])
            m1 = (C2["xy"], D1, C2["yz"])
            m2 = (C2["xz"], C2["yz"], D2)

            def cross(u, v):
                rx, ry, rz = nt2_tile(sp), nt2_tile(sp), nt2_tile(sp)
                tt(rx, u[1], v[2], ALU.mult)
                tt(T21, u[2], v[1], ALU.mult)
                tt(rx, rx, T21, ALU.subtract)
                tt(ry, u[2], v[0], ALU.mult)
                tt(T21, u[0], v[2], ALU.mult)
                tt(ry, ry, T21, ALU.subtract)
                tt(rz, u[0], v[1], ALU.mult)
                tt(T21, u[1], v[0], ALU.mult)
                tt(rz, rz, T21, ALU.subtract)
                return rx, ry, rz

            def norm2(c):
                n = nt2_tile(sp)
                sq_act(n, c[0])
                sq_act(T21, c[1])
                tt(n, n, T21, ALU.add)
                sq_act(T21, c[2])
                tt(n, n, T21, ALU.add)
                return n

            c01 = cross(m0, m1)
            c02 = cross(m0, m2)
            c12 = cross(m1, m2)
            n01, n02, n12 = norm2(c01), norm2(c02), norm2(c12)
            G1, G2, G3 = nt2_tile(sp), nt2_tile(sp), nt2_tile(sp)
            tt(G1, n01, n02, ALU.is_ge)
            tt(G2, n01, n12, ALU.is_ge)
            tt(G1, G1, G2, ALU.mult)                    # pick01
            tt(G3, n02, n12, ALU.is_ge)
            U = nt2_tile(sp)
            nc.vector.tensor_scalar(out=U[:], in0=G1[:], scalar1=-1.0, scalar2=1.0,
                                    op0=ALU.mult, op1=ALU.add)   # 1 - pick01
            tt(G2, U, G3, ALU.mult)                     # pick02
            nc.vector.tensor_scalar(out=G3[:], in0=G3[:], scalar1=-1.0, scalar2=1.0,
                                    op0=ALU.mult, op1=ALU.add)   # 1 - g3
            tt(G3, U, G3, ALU.mult)                     # pick12
            out = []
            for ci in range(3):
                VC = nt2_tile()
                tt(VC, c01[ci], G1, ALU.mult)
                tt(T21, c02[ci], G2, ALU.mult)
                tt(VC, VC, T21, ALU.add)
                tt(T21, c12[ci], G3, ALU.mult)
                tt(VC, VC, T21, ALU.add)
                out.append(VC)
            n2v = norm2(out)
            n = nt2_tile(sp)
            polished_sqrt(n, n2v, T21)
            nc.vector.tensor_scalar_max(n[:], n[:], 1e-30)
            nc.vector.reciprocal(n[:], n[:])
            for VC in out:
                tt(VC, VC, n, ALU.mult)
            return out

        EV = evec2(LAM)
        XAX = [EV[c][:, 0, :] for c in range(3)]
        ZAX = [EV[c][:, 1, :] for c in range(3)]

        # ---- disambiguation dots (batched over all tiles) ----
        SNK = [128, NT, K]

        def axis_dots(DST, AX):
            # DST[p, t, k] = sum_c NGH[p, t, cK+k] * AX[c][p, t]
            axd = sp.tile(SNK, DT.float32, tag="axd")
            for c in range(3):
                axb = _ap(AX[c]).rearrange("p t -> p t ()").to_broadcast(SNK)
                dst = DST if c == 0 else axd
                nc.vector.tensor_tensor(out=dst[:], in0=NGH[:, :, c * K:(c + 1) * K],
                                        in1=axb, op=ALU.mult)
                if c:
                    nc.vector.tensor_tensor(out=DST[:], in0=DST[:], in1=axd[:],
                                            op=ALU.add)

        DOTX = cp.tile(SNK, DT.float32)
        DOTZ = cp.tile(SNK, DT.float32)
        axis_dots(DOTX, XAX)
        axis_dots(DOTZ, ZAX)

        SG = cp.tile(SNK, DT.float32)
        FX = nt_tile()
        FZ = nt_tile()
        for DOT, F in ((DOTX, FX), (DOTZ, FZ)):
            nc.scalar.activation(SG[:], DOT[:], AF.Sign)
            nc.vector.tensor_reduce(out=F[:], in_=SG[:], axis=mybir.AxisListType.X,
                                    op=ALU.add)
            nc.scalar.activation(F[:], F[:], AF.Sign, bias=0.5, scale=1.0)
        for c in range(3):
            tt(XAX[c], XAX[c], FX, ALU.mult)
            tt(ZAX[c], ZAX[c], FZ, ALU.mult)
        fxb = FX[:].rearrange("p t -> p t ()").to_broadcast(SNK)
        nc.vector.tensor_tensor(out=DOTX[:], in0=DOTX[:], in1=fxb, op=ALU.mult)
        # y = cross(z, x)
        YAX = []
        for (i1, i2) in ((1, 2), (2, 0), (0, 1)):
            YC = nt_tile()
            tt(YC, ZAX[i1], XAX[i2], ALU.mult)
            tt(T1, ZAX[i2], XAX[i1], ALU.mult)
            tt(YC, YC, T1, ALU.subtract)
            YAX.append(YC)
        DOTY = cp.tile(SNK, DT.float32)
        axis_dots(DOTY, YAX)

        # ---- projections (batched over all tiles) ----
        PX = cp.tile([128, NT, K], DT.float32)
        PY = cp.tile([128, NT, K], DT.float32)
        SC = cp.tile([128, NT, K], DT.float32)
        nc.scalar.activation(PX[:], DOTX[:], AF.Square)
        nc.scalar.activation(PY[:], DOTY[:], AF.Square)
        U2 = cp.tile([128, NT, K], DT.float32)
        nc.vector.tensor_tensor(out=U2[:], in0=PX[:], in1=PY[:], op=ALU.add)
        nc.scalar.activation(SC[:], U2[:], AF.Sqrt)
        # one Newton step: s' = 0.5 (s + u/s) makes sqrt correctly-rounded-ish
        RCN = cp.tile([128, NT, K], DT.float32)
        nc.vector.tensor_scalar_max(RCN[:], SC[:], 1e-30)
        nc.vector.reciprocal(RCN[:], RCN[:])
        nc.vector.tensor_tensor(out=RCN[:], in0=U2[:], in1=RCN[:], op=ALU.mult)
        nc.vector.tensor_tensor(out=SC[:], in0=SC[:], in1=RCN[:], op=ALU.add)
        nc.vector.tensor_scalar(out=SC[:], in0=SC[:], scalar1=0.5, scalar2=EPS,
                                op0=ALU.mult, op1=ALU.add)
        nc.vector.reciprocal(SC[:], SC[:])
        nc.vector.tensor_tensor(out=SC[:], in0=SC[:], in1=DD[:], op=ALU.mult)
        nc.vector.tensor_tensor(out=PX[:], in0=DOTX[:], in1=SC[:], op=ALU.mult)
        nc.vector.tensor_tensor(out=PY[:], in0=DOTY[:], in1=SC[:], op=ALU.mult)

        # ---- BC selection, tiles processed in pairs: key = |p - t|^2 via
        # direct squared differences (squares on the scalar engine),
        # bit-packed with the k slot; top-3 per cell via 3 segmented
        # reduce-max + onehot-mask passes. Coordinates/weights recovered on
        # host from the slots + (PX, PY).
        MS = [cp.tile([128, NT, NCELL], DT.float32, tag=f"ms{s}", name=f"ms{s}")
              for s in range(3)]
        if _stub:
            for s in range(3):
                nc.vector.memset(MS[s][:], 0.0)
        for t in range(0, 0 if _stub else NT, 2):
            pxb = PX[:, t:t + 2, :].rearrange("p t k -> p t () k").to_broadcast(shp)
            pyb = PY[:, t:t + 2, :].rearrange("p t k -> p t () k").to_broadcast(shp)
            TA = bp.tile(shp, DT.float32, tag="ta")
            TB = bp.tile(shp, DT.float32, tag="tb")
            nc.vector.tensor_tensor(out=TA[:], in0=TXP4[:], in1=pxb, op=ALU.subtract)
            nc.vector.tensor_tensor(out=TB[:], in0=TYP4[:], in1=pyb, op=ALU.subtract)
            nc.scalar.activation(TA[:], TA[:], AF.Square)
            nc.scalar.activation(TB[:], TB[:], AF.Square)
            nc.vector.tensor_tensor(out=TA[:], in0=TA[:], in1=TB[:], op=ALU.add)
            NKEY = bp.tile(shp, DT.float32, tag="tc")
            nc.vector.scalar_tensor_tensor(
                out=NKEY[:].bitcast(DT.int32), in0=TA[:].bitcast(DT.int32),
                scalar=M32[:], in1=KIOTA2[:], op0=ALU.bitwise_and,
                op1=ALU.bitwise_or)
            srcv = NKEY
            for s in range(3):
                nc.vector.tensor_reduce(out=MS[s][:, t:t + 2, :], in_=srcv[:],
                                        axis=mybir.AxisListType.X, op=ALU.max)
                if s == 2:
                    break
                msb = MS[s][:, t:t + 2, :].rearrange("p t c -> p t c ()") \
                    .to_broadcast(shp)
                OH = bp.tile(shp, DT.float32, tag=("tb" if s == 0 else "ta"))
                nc.vector.tensor_tensor(out=OH[:], in0=srcv[:], in1=msb,
                                        op=ALU.is_equal)
                NKN = bp.tile(shp, DT.float32, tag=("nk2" if s == 0 else "nk3"))
                if s == 0:
                    nc.vector.scalar_tensor_tensor(
                        out=NKN[:], in0=OH[:], scalar=-1e30, in1=srcv[:],
                        op0=ALU.mult, op1=ALU.add)
                else:
                    OHS = bp.tile(shp, DT.float32, tag="tb")
                    nc.gpsimd.tensor_tensor(out=OHS[:], in0=OH[:], in1=NEGB[:],
                                            op=ALU.mult)
                    nc.vector.tensor_tensor(out=NKN[:], in0=OHS[:], in1=srcv[:],
                                            op=ALU.add)
                srcv = NKN
        for s in range(3):
            nc.sync.dma_start(
                m3_o[s].rearrange("(t p) c -> p t c", p=128), MS[s][:])
        nc.sync.dma_start(px_o[:].rearrange("(t p) k -> p t k", p=128), PX[:])
        nc.sync.dma_start(py_o[:].rearrange("(t p) k -> p t k", p=128), PY[:])

    split_sync_waits(nc)
    return nc


# ---------------------------------------------------------------------------
# Host glue
# ---------------------------------------------------------------------------


def _fp16_split(x):
    hi = x.astype(np.float16)
    lo = (x - hi.astype(f32)).astype(np.float16)
    return hi, lo


def host_prep_phase1(vertices):
    """vertices (4, 4096, 3) -> list of 8 input maps (fp16 hi/lo GEMM rows)."""
    maps = []
    for core in range(8):
        b, h = core // 2, core % 2
        verts = np.ascontiguousarray(vertices[b], dtype=f32)
        p2 = (verts * verts).sum(-1, dtype=f32)
        ph, pl = _fp16_split(verts.T)
        p2h, p2l = _fp16_split(p2[None, :])
        # moving rows pair with stationary rows [qh, ql, qh, 1, 1]
        ptm = np.ascontiguousarray(np.concatenate([ph, ph, pl, p2h, p2l], 0))
        Q = verts[h * HALF:(h + 1) * HALF]
        qh, ql = _fp16_split(-2.0 * Q.T)
        ones = np.ones((2, HALF), np.float16)
        qtm = np.ascontiguousarray(np.concatenate([qh, ql, qh, ones], 0))
        q2 = (Q * Q).sum(-1, dtype=f32)
        q2v = np.ascontiguousarray(q2.reshape(NT, 128).T)  # [p, t]
        maps.append({"ptm": ptm, "qtm": qtm, "q2v": q2v})
    return maps


def host_merge(cand, verts, Q):
    """Decode packed keys, exact-merge. -> nbr (HALF,32) int64, d (HALF,32), radius (HALF,)."""
    keys = np.ascontiguousarray(cand).view(np.uint32).reshape(HALF, NCH1, 8)
    gidx = (keys & np.uint32(0x7F)).astype(np.int64) + \
        (np.arange(NCH1, dtype=np.int64) * 128)[None, :, None]
    flatk = keys.reshape(HALF, CAND1)
    flati = gidx.reshape(HALF, CAND1)
    o = np.argsort(flatk, axis=1, kind="stable")[:, :33]
    idx33 = np.take_along_axis(flati, o, axis=1)
    diff = verts[idx33] - Q[:, None, :]
    d33 = np.sqrt((diff * diff).sum(-1, dtype=f32)).astype(f32)
    return idx33[:, :32], d33[:, :32], d33[:, 32]


def host_prep_phase2(vertices, template, p1_results):
    """Build phase-2 input maps + per-core nbr tables from phase-1 outputs."""
    template = np.asarray(template, f32)
    tx = template[..., 0].reshape(-1).astype(f32)
    ty = template[..., 1].reshape(-1).astype(f32)
    row = np.concatenate([tx, ty]).astype(f32)
    txy = np.ascontiguousarray(np.broadcast_to(row[None, :], (128, 2 * NCELL)))
    maps, nbrs = [], []
    for core in range(8):
        b, h = core // 2, core % 2
        verts = np.ascontiguousarray(vertices[b], dtype=f32)
        Q = verts[h * HALF:(h + 1) * HALF]
        nbr, d, radius = host_merge(p1_results[core]["cand"], verts, Q)
        neigh = (verts[nbr] - Q[:, None, :]).astype(f32)          # (HALF, 32, 3)
        ngh = np.ascontiguousarray(neigh.transpose(0, 2, 1).reshape(HALF, 96))
        w = (radius[:, None] - d).astype(f32)
        wn = (w / (w.sum(1, keepdims=True, dtype=f32) + f32(EPS))).astype(f32)
        nw = np.ascontiguousarray(ngh * np.tile(wn, (1, 3)))
        maps.append({"ngh": ngh, "nw": nw, "dd": np.ascontiguousarray(d),
                     "txy": txy})
        nbrs.append(nbr)
    return maps, nbrs


def host_assemble(p2_results, nbrs, template):
    """Decode slots, gather projections, barycentric weights, assemble output."""
    template = np.asarray(template, np.float64)
    tx = template[..., 0].reshape(-1)
    ty = template[..., 1].reshape(-1)
    out = np.zeros((B, V, R, A, 3, 2), f32)
    rows = np.arange(HALF)[:, None, None]
    for core in range(8):
        b, h = core // 2, core % 2
        m3 = np.ascontiguousarray(
            p2_results[core]["m3o"].transpose(1, 2, 0))           # (HALF, 40, 3)
        k3 = (m3.view(np.int32) & 31).astype(np.int64)            # (HALF, 40, 3)
        px = p2_results[core]["pxo"].astype(np.float64)           # (HALF, 32)
        py = p2_results[core]["pyo"].astype(np.float64)
        gx = px[rows, k3]                                         # (HALF, 40, 3)
        gy = py[rows, k3]
        p0x, p1x, p2x = gx[..., 0], gx[..., 1], gx[..., 2]
        p0y, p1y, p2y = gy[..., 0], gy[..., 1], gy[..., 2]
        v0x, v0y = p2x - p0x, p2y - p0y
        v1x, v1y = p1x - p0x, p1y - p0y
        v2x, v2y = tx[None, :] - p0x, ty[None, :] - p0y
        d00 = v0x * v0x + v0y * v0y
        d01 = v0x * v1x + v0y * v1y
        d02 = v0x * v2x + v0y * v2y
        d11 = v1x * v1x + v1y * v1y
        d12 = v1x * v2x + v1y * v2y
        den = d00 * d11 - d01 * d01 + 1e-6
        w2 = (d11 * d02 - d01 * d12) / den
        w1 = (d00 * d12 - d01 * d02) / den
        w0 = 1.0 - w2 - w1
        weights = np.stack([w2, w1, w0], axis=-1)                 # (HALF, 40, 3)
        nbr = nbrs[core]                                          # (HALF, 32)
        pidx = nbr[rows, k3]                                      # (HALF, 40, 3)
        sl = slice(h * HALF, (h + 1) * HALF)
        out[b, sl, ..., 0] = pidx.reshape(HALF, R, A, 3).astype(f32)
        out[b, sl, ..., 1] = weights.reshape(HALF, R, A, 3).astype(f32)
    return out


_PROGS = {}


def _prog(name):
    if name not in _PROGS:
        _PROGS[name] = build_phase1() if name == "p1" else build_phase2()
    return _PROGS[name]


def run_phase1(vertices, trace=False):
    maps = host_prep_phase1(vertices)
    return run_bass_kernel_spmd(_prog("p1"), maps, list(range(8)), trace=trace)


def kernel(vertices, template, trace=False, _timing=None):
    vertices = np.asarray(vertices, f32)
    template = np.asarray(template, f32)
    r1 = run_bass_kernel_spmd(_prog("p1"), host_prep_phase1(vertices),
                              list(range(8)), trace=trace)
    maps2, nbrs = host_prep_phase2(vertices, template, r1.results)
    r2 = run_bass_kernel_spmd(_prog("p2"), maps2, list(range(8)), trace=trace)
    if _timing is not None:
        _timing["phase1"] = r1
        _timing["phase2"] = r2
        _timing["maps2"] = maps2
        _timing["nbrs"] = nbrs
    return host_assemble(r2.results, nbrs, template)


if __name__ == "__main__":
    # Phase-1 standalone check against exact numpy KNN.
    cache = np.load("/root/problem/dev_cache/ref.npz")
    vertices = cache["vertices"]
    res = run_phase1(vertices)
    nbad = 0
    for core in range(8):
        b, h = core // 2, core % 2
        verts = vertices[b].astype(f32)
        Q = verts[h * HALF:(h + 1) * HALF]
        nbr, d, rad = host_merge(res.results[core]["cand"], verts, Q)
        d2x = ((Q[:, None, :].astype(np.float64) - verts[None, :, :]) ** 2).sum(-1)
        order = np.argsort(d2x, axis=-1, kind="stable")[:, :33]
        setbad = sum(set(nbr[r]) != set(order[r, :32]) for r in range(HALF))
        radref = np.sqrt(np.take_along_axis(d2x, order[:, 32:33], 1)[:, 0])
        print(f"core {core}: rows wrong nbr-set={setbad}/2048 "
              f"max rad err={np.abs(rad - radref).max():.2e}")
        nbad += setbad
    print("total wrong-set rows:", nbad)

